# revision 4
# baseline (speedup 1.0000x reference)
"""Trainium2 Bass kernel for a device-aware top-1 MoE layer.

Strategy (expert parallelism over 8 NeuronCores):
  - Host: gate + top-1 routing, then PAIR experts (largest with smallest
    token count) so every core gets one big slot (capacity C0) and one
    small slot (C1 <= C0).  The small slot runs last, shortening the
    compute tail after the final weight DMA byte.
  - Device (SPMD, one NEFF on 8 cores): per slot
    hT = relu(w1.T @ xT + b1); yT = w2.T @ hT + b2, bf16 weights,
    fp32 PSUM accumulation, activations kept [feature, token].

Schedule notes (from trace analysis of the 68.6us baseline):
  - The kernel is HBM-bound: ~17MB/core of bf16 weights at ~350GB/s.
    The sync HWDGE queue carries ONLY the 16x ~1MB weight DMAs in exact
    PE consumption order; x / biases / outputs ride the scalar HWDGE
    queue so they never head-of-line block the weight stream (the old
    gpsimd software-DGE output write gated exit ~4us late).
  - PE HAM warmup: ~200 tiny matmuls at program start keep the PE clock
    at 2.4GHz when the first real matmul issues (cold PE runs 1.2GHz
    for ~3.4us).  Stage loops are arrival-paced (d-outer over 16 live
    PSUM accumulators) so PE idle gaps stay < the 3.4us HAM window.
  - Stage-2 output epilogues alternate ScalarE/VectorE and the yT write
    is split in two chunks fired as soon as their epilogues retire.
"""

import numpy as np
import ml_dtypes

D = 1024
H = 2048
E = 16
NCORES = 8
P = 128
DB = D // P   # 8 d-chunks
HB = H // P   # 16 h-chunks
N_WARM = 200

_program_cache = {}


def _build_program(C0, C1):
    """Trace the per-core Bass/Tile program for slot capacities (C0, C1)."""
    import concourse.tile as tile
    from concourse import bacc, mybir

    assert C0 <= 256 and C1 <= C0
    f32 = mybir.dt.float32
    bf16 = mybir.dt.bfloat16
    AF = mybir.ActivationFunctionType
    ALU = mybir.AluOpType
    CS = [C0, C1]

    nc = bacc.Bacc(
        "TRN2", target_bir_lowering=False, debug=False, num_devices=NCORES
    )
    xT = nc.dram_tensor("xT", [D, C0 + C1], bf16, kind="ExternalInput").ap()
    w1s = nc.dram_tensor("w1s", [2, D, H], bf16, kind="ExternalInput").ap()
    w2s = nc.dram_tensor("w2s", [2, H, D], bf16, kind="ExternalInput").ap()
    bs = nc.dram_tensor("bs", [P, 2 * (HB + DB)], f32, kind="ExternalInput").ap()
    yT0 = nc.dram_tensor("yT0", [D, C0], bf16, kind="ExternalOutput").ap()
    yT1 = nc.dram_tensor("yT1", [D, C1], bf16, kind="ExternalOutput").ap()
    yTs = [yT0, yT1]

    with tile.TileContext(nc) as tc:
        with (
            tc.tile_pool(name="wp", bufs=1) as wp,
            tc.tile_pool(name="xp", bufs=2) as xp,
            tc.tile_pool(name="w1p", bufs=8) as w1p,
            tc.tile_pool(name="w2p", bufs=8) as w2p,
            tc.tile_pool(name="hp", bufs=32) as hp,
            tc.tile_pool(name="bp", bufs=1) as bp,
            tc.tile_pool(name="yp", bufs=2) as yp,
            tc.tile_pool(name="ps", bufs=8, space="PSUM") as ps,
        ):
            # ---- PE warmup: release the HAM clock gate before real work.
            # PSUM tiles are bank-aligned [P,256] f32; matmuls use [:, :C].
            warm = wp.tile([P, 16], bf16, tag="warm")
            nc.vector.memset(warm[:], 1.0)
            wacc = ps.tile([P, 256], f32, tag="acc", name="wacc")
            for _ in range(N_WARM):
                nc.tensor.matmul(
                    wacc[:16, :16], lhsT=warm[:], rhs=warm[:],
                    start=True, stop=True, skip_group_check=True,
                )

            # ---- scalar HWDGE queue: biases + x (never blocks weights).
            bt = bp.tile([P, 2 * (HB + DB)], f32, tag="b")
            nc.scalar.dma_start(bt[:], bs)
            b1v = [bt[:, 0:HB], bt[:, HB:2 * HB]]
            b2v = [bt[:, 2 * HB:2 * HB + DB], bt[:, 2 * HB + DB:]]

            xT3 = xT.rearrange("(o p) c -> p o c", p=P)      # [128, 8, C0+C1]
            xts = []
            off = 0
            for s in range(2):
                xt = xp.tile([P, DB, CS[s]], bf16, tag="xT")
                nc.scalar.dma_start(xt[:], xT3[:, :, off:off + CS[s]])
                xts.append(xt)
                off += CS[s]

            w13 = [w1s[s].rearrange("(o p) h -> p o h", p=P) for s in range(2)]
            w23 = [w2s[s].rearrange("(o p) f -> p o f", p=P) for s in range(2)]
            hts = [[None] * HB for _ in range(2)]

            def epilogue(i, out_t, acc_t, bias_col, relu):
                """Bias (+relu) from PSUM to SBUF, alternating engines."""
                if i % 2 == 0:
                    nc.scalar.activation(
                        out_t, acc_t,
                        AF.Relu if relu else AF.Identity,
                        bias=bias_col,
                    )
                elif relu:
                    nc.vector.tensor_scalar(
                        out_t, acc_t, bias_col, 0.0, ALU.add, ALU.max
                    )
                else:
                    nc.vector.tensor_scalar_add(out_t, acc_t, bias_col)

            W1G = 2   # d-chunks per w1 DMA (1MB)
            W2G = 4   # h-chunks per w2 DMA (1MB)
            for s in range(2):
                C = CS[s]
                # ---- sync queue: w1 for this slot (pure weight stream) ----
                w1ts = []
                for g in range(DB // W1G):
                    w1t = w1p.tile([P, W1G, H], bf16, tag="w1")
                    nc.sync.dma_start(
                        w1t[:], w13[s][:, g * W1G:(g + 1) * W1G, :]
                    )
                    w1ts.append(w1t)

                # ---- stage 1: hT = relu(w1.T @ xT + b1) ----
                # Two half-H passes, d-outer within each (PSUM holds 8
                # banks): the first pass consumes each arriving w1 group
                # immediately; the second pass reuses the resident tiles.
                for half in range(2):
                    accs1 = [
                        ps.tile([P, 256], f32, tag="acc",
                                name=f"acc1_{s}_{half}_{i}")
                        for i in range(HB // 2)
                    ]
                    for d in range(DB):
                        for hh in range(HB // 2):
                            h = half * (HB // 2) + hh
                            nc.tensor.matmul(
                                accs1[hh][:, :C],
                                lhsT=w1ts[d // W1G][:, d % W1G,
                                          h * P:(h + 1) * P],
                                rhs=xts[s][:, d, :],
                                start=(d == 0),
                                stop=(d == DB - 1),
                            )
                    for hh in range(HB // 2):
                        h = half * (HB // 2) + hh
                        ht = hp.tile([P, C], bf16, tag="hT")
                        epilogue(h, ht[:], accs1[hh][:, :C],
                                 b1v[s][:, h:h + 1], relu=True)
                        hts[s][h] = ht

                # ---- stage 2: yT = w2.T @ hT + b2 (h-outer: each w2 tile
                # is consumed as soon as its DMA lands) ----
                accs2 = [
                    ps.tile([P, 256], f32, tag="acc", name=f"acc2_{s}_{d}")
                    for d in range(DB)
                ]
                for g in range(HB // W2G):
                    w2t = w2p.tile([P, W2G, D], bf16, tag="w2")
                    nc.sync.dma_start(
                        w2t[:], w23[s][:, g * W2G:(g + 1) * W2G, :]
                    )
                    for hh in range(W2G):
                        h = g * W2G + hh
                        for d in range(DB):
                            nc.tensor.matmul(
                                accs2[d][:, :C],
                                lhsT=w2t[:, hh, d * P:(d + 1) * P],
                                rhs=hts[s][h][:],
                                start=(h == 0),
                                stop=(h == HB - 1),
                            )
                # Epilogues + output: two chunks, each fired on the scalar
                # HW queue as soon as its 4 epilogues retire.
                yT3 = yTs[s].rearrange("(o p) c -> p o c", p=P)
                yt = yp.tile([P, DB, C], bf16, tag="yt")
                for half in range(2):
                    for dd in range(DB // 2):
                        d = half * (DB // 2) + dd
                        epilogue(d, yt[:, d, :], accs2[d][:, :C],
                                 b2v[s][:, d:d + 1], relu=False)
                    nc.scalar.dma_start(
                        yT3[:, half * (DB // 2):(half + 1) * (DB // 2), :],
                        yt[:, half * (DB // 2):(half + 1) * (DB // 2), :],
                    )

    nc.compile()
    return nc


def kernel(x, gate_w, gate_b, w1, b1, w2, b2, _trace=False):
    from concourse.bass_utils import run_bass_kernel_spmd

    x = np.asarray(x, dtype=np.float32)
    B, S, d_in = x.shape
    T = B * S
    xf = x.reshape(T, d_in)

    # --- routing (host side: this is the dispatch/sharding step) ---
    logits = xf @ np.asarray(gate_w, dtype=np.float32) + np.asarray(
        gate_b, dtype=np.float32
    )
    top1 = np.argmax(logits, axis=-1)
    idxs = [np.nonzero(top1 == e)[0] for e in range(E)]
    counts = np.array([len(i) for i in idxs])

    # Pair the largest expert with the smallest so each core carries one
    # big slot and one small slot; the small slot runs last on-device.
    order = np.argsort(-counts, kind="stable")
    big, small = order[:NCORES], order[NCORES:][::-1]

    def cap(n):
        c = max(32, int(n))
        c = (c + 3) // 4 * 4
        return min(c, 256)

    C0 = cap(counts[big].max())
    C1 = cap(counts[small].max())
    assert counts[big].max() <= C0 and counts[small].max() <= C1, (
        "expert capacity overflow"
    )

    if (C0, C1) not in _program_cache:
        _program_cache[(C0, C1)] = _build_program(C0, C1)
    nc = _program_cache[(C0, C1)]

    bf16 = ml_dtypes.bfloat16
    w1 = np.asarray(w1)
    w2 = np.asarray(w2)
    b1 = np.asarray(b1, dtype=np.float32)
    b2 = np.asarray(b2, dtype=np.float32)
    CS = [C0, C1]

    in_maps = []
    core_experts = [(int(big[c]), int(small[c])) for c in range(NCORES)]
    for core in range(NCORES):
        xT = np.zeros((D, C0 + C1), dtype=bf16)
        w1s = np.empty((2, D, H), dtype=bf16)
        w2s = np.empty((2, H, D), dtype=bf16)
        bs = np.empty((P, 2 * (HB + DB)), dtype=np.float32)
        off = 0
        for s in range(2):
            e = core_experts[core][s]
            idx = idxs[e]
            if len(idx):
                xT[:, off:off + len(idx)] = xf[idx].T.astype(bf16)
            w1s[s] = w1[e].astype(bf16)
            w2s[s] = w2[e].astype(bf16)
            bs[:, s * HB:(s + 1) * HB] = b1[e].reshape(HB, P).T
            bs[:, 2 * HB + s * DB:2 * HB + (s + 1) * DB] = (
                b2[e].reshape(DB, P).T
            )
            off += CS[s]
        in_maps.append({"xT": xT, "w1s": w1s, "w2s": w2s, "bs": bs})

    res = run_bass_kernel_spmd(
        nc, in_maps, core_ids=list(range(NCORES)), trace=_trace
    )

    out = np.zeros((T, D), dtype=np.float32)
    for core in range(NCORES):
        for s in range(2):
            e = core_experts[core][s]
            idx = idxs[e]
            if len(idx):
                yT_out = res.results[core]["yT0" if s == 0 else "yT1"]
                out[idx] = yT_out[:, :len(idx)].T.astype(np.float32)
    if _trace:
        kernel.last_result = res
    return out.reshape(B, S, D)


# revision 5
# speedup vs baseline: 1.0202x; 1.0202x over previous
"""Trainium2 Bass kernel for a device-aware top-1 MoE layer.

Strategy (expert parallelism over 8 NeuronCores):
  - Host: gate + top-1 routing, then PAIR experts (largest with smallest
    token count) so every core gets one big slot (capacity C0) and one
    small slot (C1 <= C0).  The small slot runs last, shortening the
    compute tail after the final weight DMA byte.
  - Device (SPMD, one NEFF on 8 cores): per slot
    hT = relu(w1.T @ xT + b1); yT = w2.T @ hT + b2, bf16 weights,
    fp32 PSUM accumulation, activations kept [feature, token].

Schedule notes (from trace analysis):
  - HBM-bound: ~17MB/core of bf16 weights at ~350GB/s.  ONE in-order
    sync HWDGE queue carries everything in exact consumption order:
    bias, x0, w1(s0) g0, x1, w1(s0) g1-3, w2(s0), w1(s1), w2(s1), then
    the output writes last (they ride the then-idle queue; a second
    HWDGE queue gets starved by the weight stream and its semaphores
    can head-of-line block weight triggers).
  - x / y use partition-major DRAM layouts ([P, DB, C]) so DMA runs are
    ~2.4KB instead of 296B row slices.
  - PE HAM warmup: tiny matmuls at program start keep the PE clock at
    2.4GHz when the first real matmul issues (a cold PE runs 1.2GHz for
    ~3.4us, and stage loops are arrival-paced so PE idle gaps must stay
    under the 3.4us HAM re-throttle window).
  - Stage-2 epilogues alternate ScalarE/VectorE; each yT is written in
    two chunks so the first fires while the second half drains.
"""

import numpy as np
import ml_dtypes

D = 1024
H = 2048
E = 16
NCORES = 8
P = 128
DB = D // P   # 8 d-chunks
HB = H // P   # 16 h-chunks
N_WARM = 170

_program_cache = {}


def _build_program(C0, C1):
    """Trace the per-core Bass/Tile program for slot capacities (C0, C1)."""
    import concourse.tile as tile
    from concourse import bacc, mybir

    assert C0 <= 256 and C1 <= C0
    f32 = mybir.dt.float32
    bf16 = mybir.dt.bfloat16
    AF = mybir.ActivationFunctionType
    ALU = mybir.AluOpType
    CS = [C0, C1]

    nc = bacc.Bacc(
        "TRN2", target_bir_lowering=False, debug=False, num_devices=NCORES
    )
    xT0 = nc.dram_tensor("xT0", [P, DB, C0], bf16, kind="ExternalInput").ap()
    xT1 = nc.dram_tensor("xT1", [P, DB, C1], bf16, kind="ExternalInput").ap()
    w1s = nc.dram_tensor("w1s", [2, D, H], bf16, kind="ExternalInput").ap()
    w2s = nc.dram_tensor("w2s", [2, H, D], bf16, kind="ExternalInput").ap()
    bs = nc.dram_tensor("bs", [P, 2 * (HB + DB)], f32, kind="ExternalInput").ap()
    yT0 = nc.dram_tensor("yT0", [P, DB, C0], bf16, kind="ExternalOutput").ap()
    yT1 = nc.dram_tensor("yT1", [P, DB, C1], bf16, kind="ExternalOutput").ap()
    xTs = [xT0, xT1]
    yTs = [yT0, yT1]

    with tile.TileContext(nc) as tc:
        with (
            tc.tile_pool(name="wp", bufs=1) as wp,
            tc.tile_pool(name="xp", bufs=2) as xp,
            tc.tile_pool(name="w1p", bufs=8) as w1p,
            tc.tile_pool(name="w2p", bufs=8) as w2p,
            tc.tile_pool(name="hp", bufs=32) as hp,
            tc.tile_pool(name="bp", bufs=1) as bp,
            tc.tile_pool(name="yp", bufs=2) as yp,
            tc.tile_pool(name="ps", bufs=8, space="PSUM") as ps,
        ):
            # ---- PE warmup: release the HAM clock gate before real work.
            warm = wp.tile([P, 16], bf16, tag="warm")
            nc.vector.memset(warm[:], 1.0)
            wacc = ps.tile([P, 256], f32, tag="acc", name="wacc")
            for _ in range(N_WARM):
                nc.tensor.matmul(
                    wacc[:16, :16], lhsT=warm[:], rhs=warm[:],
                    start=True, stop=True, skip_group_check=True,
                )

            W1G = 2   # d-chunks per w1 DMA (1MB)
            W2G = 4   # h-chunks per w2 DMA (1MB)
            w13 = [w1s[s].rearrange("(o p) h -> p o h", p=P) for s in range(2)]
            w23 = [w2s[s].rearrange("(o p) f -> p o f", p=P) for s in range(2)]

            # ---- head of the sync queue: bias, x0, w1(s0) g0, x1 ----
            bt = bp.tile([P, 2 * (HB + DB)], f32, tag="b")
            nc.sync.dma_start(bt[:], bs)
            b1v = [bt[:, 0:HB], bt[:, HB:2 * HB]]
            b2v = [bt[:, 2 * HB:2 * HB + DB], bt[:, 2 * HB + DB:]]

            xts = []
            w1ts = [[], []]
            for s in range(2):
                xt = xp.tile([P, DB, CS[s]], bf16, tag="xT")
                nc.sync.dma_start(xt[:], xTs[s])
                xts.append(xt)
                if s == 0:
                    w1t = w1p.tile([P, W1G, H], bf16, tag="w1")
                    nc.sync.dma_start(w1t[:], w13[0][:, 0:W1G, :])
                    w1ts[0].append(w1t)

            hts = [[None] * HB for _ in range(2)]
            yts = []

            def epilogue(i, out_t, acc_t, bias_col, relu):
                """Bias (+relu) from PSUM to SBUF, alternating engines."""
                if i % 2 == 0:
                    nc.scalar.activation(
                        out_t, acc_t,
                        AF.Relu if relu else AF.Identity,
                        bias=bias_col,
                    )
                elif relu:
                    nc.vector.tensor_scalar(
                        out_t, acc_t, bias_col, 0.0, ALU.add, ALU.max
                    )
                else:
                    nc.vector.tensor_scalar_add(out_t, acc_t, bias_col)

            for s in range(2):
                C = CS[s]
                for g in range(0 if s else 1, DB // W1G):
                    w1t = w1p.tile([P, W1G, H], bf16, tag="w1")
                    nc.sync.dma_start(
                        w1t[:], w13[s][:, g * W1G:(g + 1) * W1G, :]
                    )
                    w1ts[s].append(w1t)

                # ---- stage 1: hT = relu(w1.T @ xT + b1) ----
                # Two half-H passes, d-outer within each (PSUM holds 8
                # banks): the first pass consumes each arriving w1 group
                # immediately; the second pass reuses the resident tiles.
                for half in range(2):
                    accs1 = [
                        ps.tile([P, 256], f32, tag="acc",
                                name=f"acc1_{s}_{half}_{i}")
                        for i in range(HB // 2)
                    ]
                    for d in range(DB):
                        for hh in range(HB // 2):
                            h = half * (HB // 2) + hh
                            nc.tensor.matmul(
                                accs1[hh][:, :C],
                                lhsT=w1ts[s][d // W1G][:, d % W1G,
                                             h * P:(h + 1) * P],
                                rhs=xts[s][:, d, :],
                                start=(d == 0),
                                stop=(d == DB - 1),
                            )
                    for hh in range(HB // 2):
                        h = half * (HB // 2) + hh
                        ht = hp.tile([P, C], bf16, tag="hT")
                        epilogue(h, ht[:], accs1[hh][:, :C],
                                 b1v[s][:, h:h + 1], relu=True)
                        hts[s][h] = ht

                # ---- stage 2: yT = w2.T @ hT + b2 (h-outer: each w2 tile
                # is consumed as soon as its DMA lands) ----
                accs2 = [
                    ps.tile([P, 256], f32, tag="acc", name=f"acc2_{s}_{d}")
                    for d in range(DB)
                ]
                for g in range(HB // W2G):
                    w2t = w2p.tile([P, W2G, D], bf16, tag="w2")
                    nc.sync.dma_start(
                        w2t[:], w23[s][:, g * W2G:(g + 1) * W2G, :]
                    )
                    for hh in range(W2G):
                        h = g * W2G + hh
                        for d in range(DB):
                            nc.tensor.matmul(
                                accs2[d][:, :C],
                                lhsT=w2t[:, hh, d * P:(d + 1) * P],
                                rhs=hts[s][h][:],
                                start=(h == 0),
                                stop=(h == HB - 1),
                            )
                yt = yp.tile([P, DB, C], bf16, tag="yt")
                for d in range(DB):
                    epilogue(d, yt[:, d, :], accs2[d][:, :C],
                             b2v[s][:, d:d + 1], relu=False)
                yts.append(yt)

            # ---- output writes LAST on the sync queue (idle by then);
            # two chunks per slot so the first overlaps the second's
            # epilogues.
            for s in range(2):
                for half in range(2):
                    sl = slice(half * (DB // 2), (half + 1) * (DB // 2))
                    nc.sync.dma_start(yTs[s][:, sl, :], yts[s][:, sl, :])

    nc.compile()
    return nc


def kernel(x, gate_w, gate_b, w1, b1, w2, b2, _trace=False):
    from concourse.bass_utils import run_bass_kernel_spmd

    x = np.asarray(x, dtype=np.float32)
    B, S, d_in = x.shape
    T = B * S
    xf = x.reshape(T, d_in)

    # --- routing (host side: this is the dispatch/sharding step) ---
    logits = xf @ np.asarray(gate_w, dtype=np.float32) + np.asarray(
        gate_b, dtype=np.float32
    )
    top1 = np.argmax(logits, axis=-1)
    idxs = [np.nonzero(top1 == e)[0] for e in range(E)]
    counts = np.array([len(i) for i in idxs])

    # Pair the largest expert with the smallest so each core carries one
    # big slot and one small slot; the small slot runs last on-device.
    order = np.argsort(-counts, kind="stable")
    big, small = order[:NCORES], order[NCORES:][::-1]

    def cap(n):
        c = max(32, int(n))
        c = (c + 3) // 4 * 4
        return min(c, 256)

    C0 = cap(counts[big].max())
    C1 = cap(counts[small].max())
    assert counts[big].max() <= C0 and counts[small].max() <= C1, (
        "expert capacity overflow"
    )

    if (C0, C1) not in _program_cache:
        _program_cache[(C0, C1)] = _build_program(C0, C1)
    nc = _program_cache[(C0, C1)]

    bf16 = ml_dtypes.bfloat16
    w1 = np.asarray(w1)
    w2 = np.asarray(w2)
    b1 = np.asarray(b1, dtype=np.float32)
    b2 = np.asarray(b2, dtype=np.float32)
    CS = [C0, C1]

    in_maps = []
    core_experts = [(int(big[c]), int(small[c])) for c in range(NCORES)]
    for core in range(NCORES):
        m = {}
        w1m = np.empty((2, D, H), dtype=bf16)
        w2m = np.empty((2, H, D), dtype=bf16)
        bs = np.empty((P, 2 * (HB + DB)), dtype=np.float32)
        for s in range(2):
            e = core_experts[core][s]
            idx = idxs[e]
            # [P, DB, C]: partition-major x so DMA runs are contiguous.
            xTm = np.zeros((P, DB, CS[s]), dtype=bf16)
            if len(idx):
                xTm[:, :, :len(idx)] = (
                    xf[idx].T.astype(bf16).reshape(DB, P, len(idx))
                    .transpose(1, 0, 2)
                )
            m[f"xT{s}"] = xTm
            w1m[s] = w1[e].astype(bf16)
            w2m[s] = w2[e].astype(bf16)
            bs[:, s * HB:(s + 1) * HB] = b1[e].reshape(HB, P).T
            bs[:, 2 * HB + s * DB:2 * HB + (s + 1) * DB] = (
                b2[e].reshape(DB, P).T
            )
        m["w1s"] = w1m
        m["w2s"] = w2m
        m["bs"] = bs
        in_maps.append(m)

    res = run_bass_kernel_spmd(
        nc, in_maps, core_ids=list(range(NCORES)), trace=_trace
    )

    out = np.zeros((T, D), dtype=np.float32)
    for core in range(NCORES):
        for s in range(2):
            e = core_experts[core][s]
            idx = idxs[e]
            if len(idx):
                yTm = res.results[core][f"yT{s}"]  # [P, DB, C]
                out[idx] = (
                    yTm[:, :, :len(idx)].transpose(1, 0, 2)
                    .reshape(D, len(idx)).T.astype(np.float32)
                )
    if _trace:
        kernel.last_result = res
    return out.reshape(B, S, D)
